# revision 7
# baseline (speedup 1.0000x reference)
"""EnergyTransformer Trainium2 kernel — 8-core data parallel (1 graph/core).

Self-contained: host-side numpy marshals/shards inputs, device program does
encoder -> correlation (conv x adj) -> 24-step energy descent -> decode.
"""

import os
import sys

sys.path.insert(0, "/opt/trn_rl_repo")

import numpy as np
import ml_dtypes

import concourse.bass as bass
import concourse.mybir as mybir
import concourse.tile as tile
from concourse import bacc, bass_utils
from concourse.masks import make_identity

F32 = mybir.dt.float32
F32R = mybir.dt.float32r
BF16 = mybir.dt.bfloat16

# model dims
D = 768
H = 12
HD = 64
HID = 3072
DEPTH = int(os.environ.get("ET_DEPTH", "12"))
NBLOCK = int(os.environ.get("ET_BLOCKS", "2"))
ALPHA = 0.1
EPS = 1e-5
BETA = 1.0 / float(np.sqrt(HD))
T = 256
N_TOK = 255
B = 8
N_CORES = 8
RB = 2      # token row-blocks of 128
DC = 6      # d chunks of 128
HC = 24     # hid chunks of 128

AF = mybir.ActivationFunctionType
ALU = mybir.AluOpType
AX = mybir.AxisListType

_CACHE = {}


# ---------------------------------------------------------------- device code

def _emit_program(debug_outs=()):
    nc = bacc.Bacc("TRN2", target_bir_lowering=False, debug=False,
                   num_devices=N_CORES)

    def din(name, shape, dt):
        return nc.dram_tensor(name, list(shape), dt, kind="ExternalInput").ap()

    def dout(name, shape, dt):
        return nc.dram_tensor(name, list(shape), dt, kind="ExternalOutput").ap()

    # per-core inputs
    XTp = din("XTp", [64, T], F32)
    PTi = din("PTi", [64, T], F32)
    ATp = din("ATp", [8, RB, 128, T], F32)        # AT_k[j,i] planes
    # shared weights
    Wenc = din("Wenc", [64, D], F32)
    Wpos = din("Wpos", [64, D], F32)
    Bfull = din("Bfull", [RB, 128, D], F32)
    LTm = din("LTm", [3, 12, RB, 128, T], F32)   # banded conv matrices
    WadjR = din("WadjR", [128, 96], F32)
    BadjR = din("BadjR", [128, 12], F32)
    WahR = din("WahR", [128, 12], F32)
    GamR = din("GamR", [128, NBLOCK], F32)
    DeltaR = din("DeltaR", [NBLOCK, 128, D], BF16)
    BdecT = din("BdecT", [64, 1], F32)
    Wdec6 = din("Wdec6", [DC, 128, 64], F32)
    WQd = din("WQd", [NBLOCK, DC, 128, D], BF16)   # [d, hz] chunks
    WKd = din("WKd", [NBLOCK, DC, 128, D], BF16)
    WQn = din("WQn", [NBLOCK, DC, 128, D], BF16)   # [hz, d] chunks
    WKn = din("WKn", [NBLOCK, DC, 128, D], BF16)
    WhnI = din("WhnI", [NBLOCK, DC, 128, HID], BF16)
    WhnTI = din("WhnTI", [NBLOCK, HC, 128, D], BF16)

    YT = dout("YT", [64, T], F32)
    AH = dout("AH", [RB * 128, T], F32)
    dbg = {}
    for dname, shape in debug_outs:
        dbg[dname] = dout(dname, list(shape), F32)

    fr = lambda ap: ap  # plain fp32 matmuls in setup path

    with tile.TileContext(nc) as tc:
        persist_cm = tc.tile_pool(name="persist", bufs=1)
        pp = persist_cm.__enter__()

        xs = [pp.tile([128, D], F32, name=f"x{m}") for m in range(RB)]
        corr = [[pp.tile([128, T], BF16, name=f"corr{h}_{rb}")
                 for rb in range(RB)] for h in range(H)]
        id_bf = pp.tile([128, 128], BF16, name="id_bf")
        id_fr = pp.tile([128, 128], F32, name="id_fr")
        gam_sb = pp.tile([128, NBLOCK], F32, name="gam_sb")
        bdect_sb = pp.tile([64, 1], F32, name="bdect_sb")
        ah_acc = [pp.tile([128, T], F32, name=f"ahat{rb}") for rb in range(RB)]

        make_identity(nc, id_bf[:])
        make_identity(nc, id_fr[:])
        nc.sync.dma_start(gam_sb[:], GamR)
        nc.sync.dma_start(bdect_sb[:], BdecT)

        # ---------------- setup: encoder, gram, conv, adj, corr, ahat
        with tc.tile_pool(name="setup", bufs=1) as sp:
            xtp_sb = sp.tile([64, T], F32, name="xtp_sb")
            pt_sb = sp.tile([64, T], F32, name="pt_sb")
            wenc_sb = sp.tile([64, D], F32, name="wenc_sb")
            wpos_sb = sp.tile([64, D], F32, name="wpos_sb")
            nc.sync.dma_start(xtp_sb[:], XTp)
            nc.sync.dma_start(pt_sb[:], PTi)
            nc.sync.dma_start(wenc_sb[:], Wenc)
            nc.sync.dma_start(wpos_sb[:], Wpos)
            bf_sb = [sp.tile([128, D], F32, name=f"bfull{m}") for m in range(RB)]
            for m in range(RB):
                nc.sync.dma_start(bf_sb[m][:], Bfull[m])
            at_sb = [[sp.tile([128, T], F32, name=f"at{k}_{jb}")
                      for jb in range(RB)] for k in range(8)]
            for k in range(8):
                for jb in range(RB):
                    nc.sync.dma_start(at_sb[k][jb][:], ATp[k, jb])
            wadj_sb = sp.tile([128, 96], F32, name="wadj_sb")
            badj_sb = sp.tile([128, 12], F32, name="badj_sb")
            wah_sb = sp.tile([128, 12], F32, name="wah_sb")
            nc.sync.dma_start(wadj_sb[:], WadjR)
            nc.sync.dma_start(badj_sb[:], BadjR)
            nc.sync.dma_start(wah_sb[:], WahR)

            xT = [sp.tile([128, T], F32, name=f"xT{k}") for k in range(DC)]
            G = [sp.tile([128, T + 2], F32, name=f"G{rb}") for rb in range(RB)]

            with tc.tile_pool(name="psA", bufs=2, space="PSUM") as psA:
                # encoder: x = XT^T@Wenc + PT^T@Wpos + Bfull
                for m in range(RB):
                    eps_t = psA.tile([128, D], F32, name="sps")
                    for nsl in (slice(0, 512), slice(512, 768)):
                        nc.tensor.matmul(eps_t[:, nsl],
                                         xtp_sb[:, m * 128:(m + 1) * 128],
                                         wenc_sb[:, nsl], start=True, stop=False)
                        nc.tensor.matmul(eps_t[:, nsl],
                                         pt_sb[:, m * 128:(m + 1) * 128],
                                         wpos_sb[:, nsl], start=False, stop=True)
                    nc.vector.tensor_add(xs[m][:], eps_t[:], bf_sb[m][:])

                # xT via PE transposes
                for k in range(DC):
                    tp = psA.tile([128, D], F32, name="sps")
                    tpr = tp[:, 0:T]
                    for m in range(RB):
                        nc.tensor.transpose(
                            tpr[:, m * 128:(m + 1) * 128],
                            fr(xs[m][:, k * 128:(k + 1) * 128]), id_fr[:])
                    nc.vector.tensor_copy(xT[k][:], tp[:, 0:T])

                # gram -> G tiles with guard cols
                for rb in range(RB):
                    nc.vector.memset(G[rb][:], 0.0)
                for m in range(RB):
                    gps = psA.tile([128, D], F32, name="sps")
                    for k in range(DC):
                        nc.tensor.matmul(gps[:, 0:T],
                                         fr(xT[k][:, m * 128:(m + 1) * 128]),
                                         fr(xT[k][:]), start=(k == 0),
                                         stop=(k == DC - 1))
                    nc.vector.tensor_copy(G[m][:, 1:T + 1], gps[:, 0:T])

            if "xsetup" in dbg:
                for m in range(RB):
                    nc.sync.dma_start(dbg["xsetup"][m * 128:(m + 1) * 128, :],
                                      xs[m][:])
            if "gram" in dbg:
                for m in range(RB):
                    nc.sync.dma_start(dbg["gram"][m * 128:(m + 1) * 128, :],
                                      G[m][:, 1:T + 1])

            # conv + adj + edges + corr + ahat, 2 passes x 6 channels
            with tc.tile_pool(name="ltp", bufs=1) as ltp, \
                 tc.tile_pool(name="rot", bufs=3) as rot, \
                 tc.tile_pool(name="psB", bufs=1, space="PSUM") as psB:
                for pi in range(2):
                    c0 = pi * 6
                    lt_sb = [[ltp.tile([128, 6 * T], F32, name=f"lt{b}_{kb}")
                              for kb in range(RB)] for b in range(3)]
                    for b in range(3):
                        for kb in range(RB):
                            for ci in range(6):
                                nc.sync.dma_start(
                                    lt_sb[b][kb][:, ci * T:(ci + 1) * T],
                                    LTm[b, c0 + ci, kb])
                    psC = [psB.tile([128, 2 * T], F32, name=f"convps{ci}")
                           for ci in range(6)]
                    for mb in range(RB):
                        for b in range(3):
                            for kb in range(RB):
                                lhs = fr(G[kb][:, b + mb * 128:
                                               b + mb * 128 + 128])
                                for ci in range(6):
                                    nc.tensor.matmul(
                                        psC[ci][:, mb * T:(mb + 1) * T], lhs,
                                        lt_sb[b][kb][:, ci * T:(ci + 1) * T],
                                        start=(b == 0 and kb == 0),
                                        stop=(b == 2 and kb == RB - 1))
                    for ci in range(6):
                        c = c0 + ci
                        for jb in range(RB):
                            adjt = rot.tile([128, T], F32, name="adjt")
                            nc.vector.tensor_scalar(
                                adjt[:], at_sb[0][jb][:],
                                wadj_sb[:, c:c + 1],
                                badj_sb[:, c:c + 1], ALU.mult, ALU.add)
                            for k in range(1, 8):
                                tmpa = rot.tile([128, T], F32, name="tmpa")
                                wsl = wadj_sb[:, k * 12 + c:k * 12 + c + 1]
                                if k % 2 == 0:
                                    nc.scalar.activation(
                                        tmpa[:], at_sb[k][jb][:], AF.Copy,
                                        scale=wsl)
                                else:
                                    nc.vector.tensor_scalar_mul(
                                        tmpa[:], at_sb[k][jb][:], wsl)
                                nc.vector.tensor_add(adjt[:], adjt[:], tmpa[:])
                            edg = rot.tile([128, T], F32, name="edg")
                            nc.vector.tensor_tensor(
                                edg[:], psC[ci][:, jb * T:(jb + 1) * T],
                                adjt[:], ALU.mult)
                            # ahat accumulation (over channels)
                            if c == 0:
                                nc.vector.tensor_scalar_mul(
                                    ah_acc[jb][:], edg[:], wah_sb[:, c:c + 1])
                            else:
                                tmpe = rot.tile([128, T], F32, name="tmpa")
                                nc.scalar.activation(tmpe[:], edg[:], AF.Copy,
                                                     scale=wah_sb[:, c:c + 1])
                                nc.vector.tensor_add(ah_acc[jb][:],
                                                     ah_acc[jb][:], tmpe[:])
                            # corr_h[rb][:, jb*128..] = edgesT[jb][:, rb*128..]^T
                            tpc = psB.tile([128, T], F32, name="tpsf",
                                           bufs=2)
                            for rb in range(RB):
                                nc.tensor.transpose(
                                    tpc[:, rb * 128:(rb + 1) * 128],
                                    fr(edg[:, rb * 128:(rb + 1) * 128]),
                                    id_fr[:])
                            for rb in range(RB):
                                nc.scalar.copy(
                                    corr[c][rb][:, jb * 128:(jb + 1) * 128],
                                    tpc[:, rb * 128:(rb + 1) * 128])

                # ahat output: transpose accumulated ahatT -> [i, j]
                for rb in range(RB):
                    tpa = psB.tile([128, T], F32, name="tpsf", bufs=2)
                    for jb in range(RB):
                        nc.tensor.transpose(
                            tpa[:, jb * 128:(jb + 1) * 128],
                            fr(ah_acc[jb][:, rb * 128:(rb + 1) * 128]),
                            id_fr[:])
                    aout = rot.tile([128, T], F32, name="aout")
                    nc.vector.tensor_copy(aout[:], tpa[:])
                    nc.sync.dma_start(AH[rb * 128:(rb + 1) * 128, :], aout[:])

        if "corr" in dbg:
            with tc.tile_pool(name="dbgc", bufs=2) as dp:
                for h in range(H):
                    for rb in range(RB):
                        t_ = dp.tile([128, T], F32, name="c32")
                        nc.vector.tensor_copy(t_[:], corr[h][rb][:])
                        nc.sync.dma_start(
                            dbg["corr"][(h * RB + rb) * 128:
                                        (h * RB + rb + 1) * 128, :], t_[:])

        # ---------------- descent loop
        with tc.tile_pool(name="wts", bufs=1) as wp, \
             tc.tile_pool(name="loop", bufs=1) as lp, \
             tc.tile_pool(name="psL", bufs=2, space="PSUM") as psp:

            wqd = [wp.tile([128, D], BF16, name=f"wqd{k}") for k in range(DC)]
            wkd = [wp.tile([128, D], BF16, name=f"wkd{k}") for k in range(DC)]
            wqn = [wp.tile([128, D], BF16, name=f"wqn{k}") for k in range(DC)]
            wkn = [wp.tile([128, D], BF16, name=f"wkn{k}") for k in range(DC)]
            whn = [wp.tile([128, HID], BF16, name=f"whn{k}") for k in range(DC)]
            whnt = [wp.tile([128, D], BF16, name=f"whnt{k}") for k in range(HC)]
            delta_sb = wp.tile([128, D], BF16, name="delta_sb")

            for blk in range(NBLOCK):
                for k in range(DC):
                    nc.sync.dma_start(wqd[k][:], WQd[blk, k])
                    nc.sync.dma_start(wkd[k][:], WKd[blk, k])
                    nc.sync.dma_start(wqn[k][:], WQn[blk, k])
                    nc.sync.dma_start(wkn[k][:], WKn[blk, k])
                    nc.sync.dma_start(whn[k][:], WhnI[blk, k])
                for k in range(HC):
                    nc.sync.dma_start(whnt[k][:], WhnTI[blk, k])
                nc.sync.dma_start(delta_sb[:], DeltaR[blk])

                for si in range(DEPTH):
                    _emit_step(nc, tc, lp, psp, xs, corr, id_bf, gam_sb,
                               delta_sb, wqd, wkd, wqn, wkn, whn, whnt, blk)
                    key = f"xs{blk}_{si}"
                    if key in dbg:
                        for m in range(RB):
                            nc.sync.dma_start(
                                dbg[key][m * 128:(m + 1) * 128, :], xs[m][:])

            # ---------------- decode: yT = Wdec^T @ x^T + b_dec
            wdec_sb = [lp.tile([128, 64], F32, name=f"wdec{k}")
                       for k in range(DC)]
            for k in range(DC):
                nc.sync.dma_start(wdec_sb[k][:], Wdec6[k])
            xTf = [lp.tile([128, T], F32, name=f"xTf{k}") for k in range(DC)]
            for k in range(DC):
                tp = psp.tile([128, 512], F32, name="psf")
                tpr = tp[:, 0:T]
                for m in range(RB):
                    nc.tensor.transpose(tpr[:, m * 128:(m + 1) * 128],
                                        fr(xs[m][:, k * 128:(k + 1) * 128]),
                                        id_fr[:])
                nc.vector.tensor_copy(xTf[k][:], tp[:, 0:T])
            yps = psp.tile([128, 512], F32, name="psf")
            for k in range(DC):
                nc.tensor.matmul(yps[0:64, 0:T], wdec_sb[k][:],
                                 fr(xTf[k][:]), start=(k == 0),
                                 stop=(k == DC - 1))
            yt_sb = lp.tile([64, T], F32, name="yt_sb")
            nc.vector.tensor_scalar_add(yt_sb[:], yps[0:64, 0:T],
                                        bdect_sb[:, 0:1])
            nc.sync.dma_start(YT, yt_sb[:])

        persist_cm.__exit__(None, None, None)

    nc.compile()
    return nc


def _emit_step(nc, tc, lp, psp, xs, corr, id_bf, gam_sb, delta_sb,
               wqd, wkd, wqn, wkn, whn, whnt, blk):
    """One descent step: x += ALPHA * (G1@Wq + G2@Wk + relu(g@Whn)@Whn^T)."""
    g = [lp.tile([128, D], BF16, name=f"g{m}") for m in range(RB)]
    # ---- LayerNorm (scalar gamma, vector delta)
    for m in range(RB):
        musum = lp.tile([128, 1], F32, name=f"musum{m}", bufs=2)
        nc.vector.reduce_sum(musum[:], xs[m][:], axis=AX.X)
        mu = lp.tile([128, 1], F32, name=f"mu{m}", bufs=2)
        nc.vector.tensor_scalar_mul(mu[:], musum[:], 1.0 / D)
        nmu = lp.tile([128, 1], F32, name=f"nmu{m}", bufs=2)
        nc.vector.tensor_scalar_mul(nmu[:], musum[:], -1.0 / D)
        sqb = lp.tile([128, D], BF16, name=f"sqb{m}")
        vsum = lp.tile([128, 1], F32, name=f"vsum{m}", bufs=2)
        nc.scalar.activation(sqb[:], xs[m][:], AF.Square, bias=nmu[:, 0:1],
                             accum_out=vsum[:])
        varp = lp.tile([128, 1], F32, name=f"varp{m}", bufs=2)
        nc.vector.tensor_scalar(varp[:], vsum[:], 1.0 / D, EPS, ALU.mult,
                                ALU.add)
        std = lp.tile([128, 1], F32, name=f"std{m}", bufs=2)
        nc.scalar.sqrt(std[:], varp[:])
        rstd = lp.tile([128, 1], F32, name=f"rstd{m}", bufs=2)
        nc.vector.reciprocal(rstd[:], std[:])
        gr = lp.tile([128, 1], F32, name=f"gr{m}", bufs=2)
        nc.vector.tensor_scalar_mul(gr[:], rstd[:], gam_sb[:, blk:blk + 1])
        gpre = lp.tile([128, D], BF16, name=f"gpre{m}")
        nc.vector.tensor_scalar(gpre[:], xs[m][:], mu[:, 0:1], gr[:, 0:1],
                                ALU.subtract, ALU.mult)
        nc.vector.tensor_add(g[m][:], gpre[:], delta_sb[:])

    # ---- gT transposes: gT[k][:, m*128..] = g[m][:, k*128..]^T
    gT = [lp.tile([128, T], BF16, name=f"gT{k}") for k in range(DC)]
    for kp in range(3):
        tp = psp.tile([128, 512], BF16, name="psb")
        for j in range(2):
            k = kp * 2 + j
            for m in range(RB):
                nc.tensor.transpose(
                    tp[:, j * 256 + m * 128:j * 256 + (m + 1) * 128],
                    g[m][:, k * 128:(k + 1) * 128], id_bf[:])
        for j in range(2):
            k = kp * 2 + j
            nc.scalar.copy(gT[k][:], tp[:, j * 256:(j + 1) * 256])

    # ---- Q, K token-major
    Q = [lp.tile([128, D], BF16, name=f"Q{m}") for m in range(RB)]
    K = [lp.tile([128, D], BF16, name=f"K{m}") for m in range(RB)]
    for m in range(RB):
        for dst, w in ((Q, wqd), (K, wkd)):
            ps = psp.tile([128, D], F32, name="big")
            for k in range(DC):
                lhs = gT[k][:, m * 128:(m + 1) * 128]
                for nsl in (slice(0, 512), slice(512, 768)):
                    nc.tensor.matmul(ps[:, nsl], lhs, w[k][:, nsl],
                                     start=(k == 0), stop=(k == DC - 1))
            nc.scalar.copy(dst[m][:], ps[:])

    # ---- QT, KT (z-major) via transposes
    QT = [lp.tile([128, T], BF16, name=f"QT{k}") for k in range(DC)]
    KT = [lp.tile([128, T], BF16, name=f"KT{k}") for k in range(DC)]
    for src, dst in ((Q, QT), (K, KT)):
        for kp in range(3):
            tp = psp.tile([128, 512], BF16, name="psb")
            for j in range(2):
                k = kp * 2 + j
                for m in range(RB):
                    nc.tensor.transpose(
                        tp[:, j * 256 + m * 128:j * 256 + (m + 1) * 128],
                        src[m][:, k * 128:(k + 1) * 128], id_bf[:])
            for j in range(2):
                nc.scalar.copy(dst[kp * 2 + j][:], tp[:, j * 256:(j + 1) * 256])

    # ---- attention heads
    G1T = [lp.tile([128, T], BF16, name=f"G1T{k}") for k in range(DC)]
    G2T = [lp.tile([128, T], BF16, name=f"G2T{k}") for k in range(DC)]
    for h in range(H):
        jt, po = h // 2, (h % 2) * 64
        lps = psp.tile([128, 512], F32, name="psf")
        for rb in range(RB):
            nc.tensor.matmul(lps[:, rb * 256:(rb + 1) * 256],
                             QT[jt][po:po + 64, rb * 128:(rb + 1) * 128],
                             KT[jt][po:po + 64, :], start=True, stop=True)
        Wt = [lp.tile([128, T], BF16, name=f"Wt{rb}", bufs=2)
              for rb in range(RB)]
        for rb in range(RB):
            lb = lp.tile([128, T], BF16, name="smtmp", bufs=4)
            nc.vector.tensor_tensor(lb[:], lps[:, rb * 256:(rb + 1) * 256],
                                    corr[h][rb][:], ALU.mult)
            eb = lp.tile([128, T], BF16, name="smtmp", bufs=4)
            rsum = lp.tile([128, 1], F32, name=f"rsum{rb}", bufs=2)
            nc.scalar.activation(eb[:], lb[:], AF.Exp, scale=BETA,
                                 accum_out=rsum[:])
            rinv = lp.tile([128, 1], F32, name=f"rinv{rb}", bufs=2)
            nc.vector.reciprocal(rinv[:], rsum[:])
            ec = lp.tile([128, T], BF16, name="smtmp", bufs=4)
            nc.vector.tensor_tensor(ec[:], eb[:], corr[h][rb][:], ALU.mult)
            nc.vector.tensor_scalar_mul(Wt[rb][:], ec[:], rinv[:, 0:1])
        # WtT[:, kb*256 + qb*128 ..] = Wt[qb][:, kb*128..]^T
        WtT = lp.tile([128, 512], BF16, name="WtT", bufs=2)
        tpw = psp.tile([128, 512], BF16, name="psb")
        for kb in range(RB):
            for qb in range(RB):
                nc.tensor.transpose(
                    tpw[:, kb * 256 + qb * 128:kb * 256 + (qb + 1) * 128],
                    Wt[qb][:, kb * 128:(kb + 1) * 128], id_bf[:])
        nc.scalar.copy(WtT[:], tpw[:])
        # G1T_h = K_h^T @ Wt^T ; G2T_h = Q_h^T @ Wt  (each [64, 256])
        gps = psp.tile([128, 512], F32, name="psf")
        for kb in range(RB):
            nc.tensor.matmul(gps[0:64, 0:256], K[kb][:, h * 64:h * 64 + 64],
                             WtT[:, kb * 256:(kb + 1) * 256],
                             start=(kb == 0), stop=(kb == RB - 1))
        for rb in range(RB):
            nc.tensor.matmul(gps[0:64, 256:512], Q[rb][:, h * 64:h * 64 + 64],
                             Wt[rb][:], start=(rb == 0), stop=(rb == RB - 1))
        nc.scalar.copy(G1T[jt][po:po + 64, :], gps[0:64, 0:256])
        nc.scalar.copy(G2T[jt][po:po + 64, :], gps[0:64, 256:512])

    # ---- backward projections into grad PSUM
    gradps = []
    for m in range(RB):
        gp = psp.tile([128, D], F32, name="big")
        gradps.append(gp)
        for gsrc, w in ((G1T, wqn), (G2T, wkn)):
            for k in range(DC):
                lhs = gsrc[k][:, m * 128:(m + 1) * 128]
                for nsl in (slice(0, 512), slice(512, 768)):
                    nc.tensor.matmul(gp[:, nsl], lhs, w[k][:, nsl],
                                     start=(gsrc is G1T and k == 0),
                                     stop=False)
    # ---- hopfield forward + rhT transposes
    rhT = lp.tile([128, HC * T], BF16, name="rhT")
    rhT3 = rhT.rearrange("p (kk t) -> p kk t", kk=HC)
    for m in range(RB):
        for nh in range(6):
            hps = psp.tile([128, 512], F32, name="psf")
            for k in range(DC):
                nc.tensor.matmul(hps[:], gT[k][:, m * 128:(m + 1) * 128],
                                 whn[k][:, nh * 512:(nh + 1) * 512],
                                 start=(k == 0), stop=(k == DC - 1))
            rch = lp.tile([128, 512], BF16, name="rch", bufs=2)
            nc.scalar.activation(rch[:], hps[:], AF.Relu)
            tp = psp.tile([128, 512], BF16, name="psb")
            for q in range(4):
                nc.tensor.transpose(tp[:, q * 128:(q + 1) * 128],
                                    rch[:, q * 128:(q + 1) * 128], id_bf[:])
            nc.scalar.copy(
                rhT3[:, nh * 4:(nh + 1) * 4, m * 128:(m + 1) * 128],
                tp[:].rearrange("p (q t) -> p q t", q=4))
    # ---- hopfield backward
    for m in range(RB):
        for kk in range(HC):
            lhs = rhT3[:, kk, m * 128:(m + 1) * 128]
            for nsl in (slice(0, 512), slice(512, 768)):
                nc.tensor.matmul(gradps[m][:, nsl], lhs, whnt[kk][:, nsl],
                                 start=False, stop=(kk == HC - 1))
    # ---- x update
    for m in range(RB):
        upd = lp.tile([128, D], F32, name=f"upd{m}")
        nc.scalar.activation(upd[:], gradps[m][:], AF.Copy, scale=ALPHA)
        nc.vector.tensor_add(xs[m][:], xs[m][:], upd[:])


# ---------------------------------------------------------------- host side

def _bf(x):
    return np.ascontiguousarray(np.asarray(x).astype(ml_dtypes.bfloat16))


def _f32(x):
    return np.ascontiguousarray(np.asarray(x, dtype=np.float32))


def _prep_shared(params):
    p = {k: np.asarray(v, np.float32) for k, v in params.items()}
    out = {}
    out["Wenc"] = _f32(p["W_enc"])
    out["Wpos"] = _f32(p["W_pos"])
    bfull = np.empty((T, D), np.float32)
    bfull[:] = p["b_enc"][None, :] + p["b_pos"][None, :]
    bfull[0] = p["cls"][0] + p["b_pos"]
    out["Bfull"] = np.ascontiguousarray(bfull.reshape(RB, 128, D))
    # banded conv matrices: LT[b][c][p, i] = conv_k[p-i+1, b, 0, c]
    kk = p["conv_k"]  # [3,3,1,H]
    lt = np.zeros((3, 12, T, T), np.float32)
    idx = np.arange(T)
    for a in range(3):
        pr = idx + a - 1
        mask = (pr >= 0) & (pr < T)
        for b_ in range(3):
            for c in range(12):
                lt[b_, c, pr[mask], idx[mask]] = kk[a, b_, 0, c]
    out["LTm"] = np.ascontiguousarray(lt.reshape(3, 12, RB, 128, T))
    out["WadjR"] = np.ascontiguousarray(
        np.tile(p["W_adj"].reshape(1, 96), (128, 1)).astype(np.float32))
    out["BadjR"] = np.ascontiguousarray(
        np.tile(p["b_adj"].reshape(1, 12), (128, 1)).astype(np.float32))
    out["WahR"] = np.ascontiguousarray(
        np.tile(p["W_ah"].reshape(1, 12), (128, 1)).astype(np.float32))
    out["GamR"] = np.ascontiguousarray(
        np.tile(p["gamma"][:NBLOCK].reshape(1, NBLOCK), (128, 1))
        .astype(np.float32))
    out["DeltaR"] = _bf(np.tile(p["delta"][:NBLOCK, None, :], (1, 128, 1)))
    out["BdecT"] = _f32(p["b_dec"].reshape(64, 1))
    out["Wdec6"] = _f32(p["W_dec"].reshape(DC, 128, 64))
    wq = p["Wq"][:NBLOCK].reshape(NBLOCK, H * HD, D)
    wk = p["Wk"][:NBLOCK].reshape(NBLOCK, H * HD, D)
    out["WQd"] = _bf(np.ascontiguousarray(wq.transpose(0, 2, 1))
                     .reshape(NBLOCK, DC, 128, D))
    out["WKd"] = _bf(np.ascontiguousarray(wk.transpose(0, 2, 1))
                     .reshape(NBLOCK, DC, 128, D))
    out["WQn"] = _bf(wq.reshape(NBLOCK, DC, 128, D))
    out["WKn"] = _bf(wk.reshape(NBLOCK, DC, 128, D))
    whn_ = p["Whn"][:NBLOCK]
    out["WhnI"] = _bf(whn_.reshape(NBLOCK, DC, 128, HID))
    out["WhnTI"] = _bf(np.ascontiguousarray(whn_.transpose(0, 2, 1))
                       .reshape(NBLOCK, HC, 128, D))
    return out


def _prep_core(Xb, Ab, Pb):
    xtp = np.zeros((64, T), np.float32)
    xtp[:, 1:] = np.asarray(Xb, np.float32).T
    pt = np.ascontiguousarray(np.asarray(Pb, np.float32).T)
    atp = np.ascontiguousarray(
        np.asarray(Ab, np.float32).transpose(2, 1, 0)).reshape(8, RB, 128, T)
    return {"XTp": xtp, "PTi": pt, "ATp": atp}


def _get_program(debug_outs=()):
    key = ("prog", tuple(debug_outs))
    if key not in _CACHE:
        _CACHE[key] = _emit_program(debug_outs)
    return _CACHE[key]


def run_device(X, A, P, params, debug_outs=(), trace=False):
    nc = _get_program(tuple(debug_outs))
    shared = _prep_shared(params)
    in_maps = []
    for b in range(B):
        m = dict(shared)
        m.update(_prep_core(X[b], A[b], P[b]))
        in_maps.append(m)
    res = bass_utils.run_bass_kernel_spmd(
        nc, in_maps, core_ids=list(range(N_CORES)), trace=trace)
    return res


def kernel(X, A, P, params):
    X = np.asarray(X, np.float32)
    A = np.asarray(A, np.float32)
    P = np.asarray(P, np.float32)
    res = run_device(X, A, P, params)
    cls = np.empty((B, 1, 64), np.float32)
    xso = np.empty((B, N_TOK, 64), np.float32)
    ahs = np.empty((B, T, T, 1), np.float32)
    for b in range(B):
        y = res.results[b]["YT"].T  # [256, 64]
        cls[b] = y[:1]
        xso[b] = y[1:]
        ahs[b] = res.results[b]["AH"][:, :, None]
    return cls, xso, ahs
